# revision 30
# baseline (speedup 1.0000x reference)
"""Trainium2 Bass kernel for nn_Conv2d: x[32,128,56,56] * W[256,128,3,3] + b -> [32,256,56,56].

Stride 1, padding 1, dilation 1. Data-parallel over batch across 8 NeuronCores
(4 images per core, no collectives). Per core the conv is one accumulation
group of 9 matmuls per output tile (one per kernel tap):
PSUM[cout_chunk=128, R*56] += matmul(lhsT=Wt[tap][cin, cout_chunk],
rhs=shifted window of the zero-padded input row-block [cin=128, R+2, 58]).
Bias is fused into the PSUM->SBUF drain on the scalar engine.

Matmuls run in bf16 (1 PE cycle/row vs 4 for exact fp32; enables fast weight
load, so the per-matmul weight switch hides under the previous matmul's
streaming). PSUM accumulation and the output stay fp32; measured absmax rel
err is ~2e-3 vs the fp32 reference (tolerance 2e-2).

Hard-won scheduling facts baked in here (each measured on hardware):
- Each group's 9 matmuls must stay consecutive, on one PSUM bank, with the
  natural [128, R, 56] output AP. Flattening the AP to [128, 448] or
  alternating PSUM banks/rhs tiles between matmuls costs ~40ns on every
  matmul (~20% of its 191ns pitch).
- x row-tile DMAs are issued just-in-time inside the main loop (prefetch
  depth 5), interleaved with output DMAs. The Sync queue triggers DMAs in
  order through an 8-slot completion window, so bulk-issuing all 28 input
  tiles up front parks every output DMA behind ~8MB of input traffic ->
  output SBUF buffers never recycle -> PSUM fills -> the PE stalls mid-run
  and the HAM clock gate re-throttles it (measured 9us stall + 10us at half
  clock).
- A 10-matmul warm-up group on zeroed SBUF (result never read) runs during
  the initial DMA wait so the HAM activity monitor flips the PE to full
  clock (2.4GHz, not the cold 1.2GHz) before/just after the first real
  matmul.
- The final tile ships one DMA per cout chunk, and its last chunk runs as
  two half-height groups (same streamed columns, ~free) so the closing
  drain + DMA overlap the final matmuls: tail measured 1.0us vs 2.5us.
  Note the extra 'ph' pool tag costs ~1.1us of epilogue — the split is
  still net positive (~-0.4us), but that trade bounds how far tail
  splitting can be pushed.
- The framework epilogue serially resets every allocated semaphore
  (~115ns each, ~8.7us total) — every extra pool tag, DMA, or engine queue
  used inflates it, which is why this kernel keeps the structure minimal
  (measured: fancier variants gained ~1.4us in flight but paid it all back
  in epilogue).

Self-contained: hardcodes shapes; host-side pre-pads/retiles x and
pre-transposes W so every device DMA is contiguous.
"""

import numpy as np

B, CIN, H, W_ = 32, 128, 56, 56
COUT, KH, KW = 256, 3, 3
NCORES = 8
BPC = B // NCORES          # images per core
R = 8                      # output rows per tile -> matmul free dim R*56 = 448
NT = H // R                # row tiles per image
NTILE = BPC * NT
HP, WP = H + 2, W_ + 2     # padded 58x58
NCH = COUT // 128          # cout chunks (2)

MM_DTYPE = "bfloat16"
XBUFS = 6                  # x-tile ring depth
PREFETCH = 5               # x tiles loaded ahead of consumption

_cache = {}


def _np_mm_dtype():
    if MM_DTYPE == "bfloat16":
        try:
            import ml_dtypes

            return ml_dtypes.bfloat16
        except ImportError:
            import jax.numpy as jnp

            return np.dtype(jnp.bfloat16)
    return np.float32


def _build(mm_dtype_name):
    import concourse.mybir as mybir
    import concourse.tile as tile
    from concourse import bacc

    dt = mybir.dt
    mmdt = getattr(dt, mm_dtype_name)

    nc = bacc.Bacc("TRN2", target_bir_lowering=False, debug=False)

    # x arrives host-pre-padded per row-tile: [image, row_tile, cin, R+2, 58]
    # (zero border baked in, halo rows duplicated) so every x DMA is one
    # fully contiguous copy and the kernel needs no memsets.
    x_d = nc.dram_tensor(
        "x", [BPC, NT, CIN, R + 2, WP], mmdt, kind="ExternalInput"
    )
    # [chunk, cin, tap, cout_slice]: one contiguous DMA per cout chunk
    wt_d = nc.dram_tensor("wt", [NCH, CIN, KH * KW, 128], mmdt, kind="ExternalInput")
    b_d = nc.dram_tensor("bias", [128, NCH], dt.float32, kind="ExternalInput")
    # Output laid out [image, cout%128 (partition), cout//128, h, w] so both
    # cout chunks of one row-tile go out in a single DMA; host untangles.
    o_d = nc.dram_tensor(
        "out", [BPC, 128, NCH, H, W_], dt.float32, kind="ExternalOutput"
    )

    with tile.TileContext(nc) as tc:
        with (
            tc.tile_pool(name="const", bufs=1) as const_pool,
            tc.tile_pool(name="xin", bufs=XBUFS) as xin_pool,
            tc.tile_pool(name="outp", bufs=4) as out_pool,
            tc.tile_pool(name="psum", bufs=6, space="PSUM") as psum_pool,
        ):
            xt = []

            def load_x(idx):
                n, ht = divmod(idx, NT)
                t = xin_pool.tile([CIN, R + 2, WP], mmdt, tag="xt")
                nc.sync.dma_start(t[:], x_d[n, ht])
                xt.append(t)

            # PE clock warm-up (see module docstring).
            zw_t = const_pool.tile([CIN, 128], mmdt)
            nc.gpsimd.memset(zw_t[:], 0.0)
            zx_t = const_pool.tile([CIN, R, W_], mmdt)
            nc.gpsimd.memset(zx_t[:], 0.0)
            pw = psum_pool.tile([128, R, W_], dt.float32, tag="ps")
            for i in range(10):
                nc.tensor.matmul(
                    pw[:],
                    zw_t[:],
                    zx_t[:],
                    start=(i == 0),
                    stop=(i == 9),
                )

            # Critical path first: the first x tile (the startup gater) ships
            # as two row-slices so the kernel's first matmul group — chunk 0
            # of tile 0, run as two half-height pieces on the 'ph' tag the
            # tail split already pays for — gates on an 87KB transfer instead
            # of 148KB (subtile deps track the partial writes). Then tap-0 of
            # chunk-0 weights, the rest of the constants, and the prefetch
            # window.
            t0 = xin_pool.tile([CIN, R + 2, WP], mmdt, tag="xt")
            nc.sync.dma_start(t0[:, 0 : R - 2, :], x_d[0, 0, :, 0 : R - 2, :])
            nc.sync.dma_start(t0[:, R - 2 :, :], x_d[0, 0, :, R - 2 :, :])
            xt.append(t0)
            w_t = const_pool.tile([CIN, NCH, KH * KW, 128], mmdt)
            nc.sync.dma_start(w_t[:, 0, 0], wt_d[0, :, 0])
            nc.sync.dma_start(w_t[:, 0, 1:], wt_d[0, :, 1:])
            nc.sync.dma_start(w_t[:, 1], wt_d[1])
            b_t = const_pool.tile([128, NCH], dt.float32)
            nc.sync.dma_start(b_t[:], b_d[:])
            for i in range(1, PREFETCH):
                load_x(i)

            for idx in range(NTILE):
                n, ht = divmod(idx, NT)
                if idx + PREFETCH < NTILE:
                    load_x(idx + PREFETCH)
                t = xt[idx]
                ot = out_pool.tile([128, NCH, R, W_], dt.float32, tag="ot")
                for c in range(NCH):
                    if idx == 0 and c == 0:
                        # Mirror of the tail split at the head: the kernel's
                        # first group in two half-height pieces so matmul #1
                        # only needs the first row-slice DMA of tile 0.
                        hr = R // 2
                        for half in range(2):
                            ph = psum_pool.tile(
                                [128, hr, W_], dt.float32, tag="ph", bufs=2
                            )
                            hb = half * hr
                            for kh in range(KH):
                                for kw in range(KW):
                                    pos = kh * KW + kw
                                    nc.tensor.matmul(
                                        ph[:],
                                        w_t[:, 0, pos],
                                        t[:, hb + kh : hb + kh + hr, kw : kw + W_],
                                        start=(pos == 0),
                                        stop=(pos == KH * KW - 1),
                                    )
                            nc.scalar.activation(
                                ot[:, 0, hb : hb + hr],
                                ph[:],
                                mybir.ActivationFunctionType.Identity,
                                bias=b_t[:, 0:1],
                            )
                        continue
                    if idx == NTILE - 1 and c == 1:
                        # The very last group runs as two half-height pieces
                        # (same total streamed columns, ~free) so the final
                        # drain + output DMA overlap the last matmuls instead
                        # of serializing after them.
                        hr = R // 2
                        for half in range(2):
                            ph = psum_pool.tile(
                                [128, hr, W_], dt.float32, tag="ph", bufs=2
                            )
                            hb = half * hr
                            for kh in range(KH):
                                for kw in range(KW):
                                    pos = kh * KW + kw
                                    nc.tensor.matmul(
                                        ph[:],
                                        w_t[:, 1, pos],
                                        t[:, hb + kh : hb + kh + hr, kw : kw + W_],
                                        start=(pos == 0),
                                        stop=(pos == KH * KW - 1),
                                    )
                            nc.scalar.activation(
                                ot[:, 1, hb : hb + hr],
                                ph[:],
                                mybir.ActivationFunctionType.Identity,
                                bias=b_t[:, 1:2],
                            )
                            nc.sync.dma_start(
                                o_d[n, :, 1, ht * R + hb : ht * R + hb + hr, :],
                                ot[:, 1, hb : hb + hr],
                            )
                        continue
                    p = psum_pool.tile([128, R, W_], dt.float32, tag="ps")
                    for kh in range(KH):
                        for kw in range(KW):
                            pos = kh * KW + kw
                            nc.tensor.matmul(
                                p[:],
                                w_t[:, c, pos],
                                t[:, kh : kh + R, kw : kw + W_],
                                start=(pos == 0),
                                stop=(pos == KH * KW - 1),
                            )
                    nc.scalar.activation(
                        ot[:, c],
                        p[:],
                        mybir.ActivationFunctionType.Identity,
                        bias=b_t[:, c : c + 1],
                    )
                    if idx == NTILE - 1:
                        # Tail latency: ship each chunk of the final tile as
                        # soon as its drain finishes instead of waiting for
                        # both.
                        nc.sync.dma_start(
                            o_d[n, :, c, ht * R : ht * R + R, :],
                            ot[:, c],
                        )
                if idx < NTILE - 1:
                    nc.sync.dma_start(
                        o_d[n, :, :, ht * R : ht * R + R, :],
                        ot[:],
                    )

    nc.compile()
    return nc


def _make_in_maps(x, W, b):
    mdt = _np_mm_dtype()
    x = np.asarray(x, dtype=np.float32)
    W = np.asarray(W, dtype=np.float32)
    b = np.asarray(b, dtype=np.float32)

    # Pre-pad and re-tile x: [B, CIN, 56, 56] -> [B, NT, CIN, R+2, 58] where
    # row-tile ht holds padded rows h0..h0+R+1 (zero border baked in).
    xpad = np.zeros((B, CIN, HP, WP), dtype=mdt)
    xpad[:, :, 1 : H + 1, 1 : W_ + 1] = x.astype(mdt)
    xt = np.empty((B, NT, CIN, R + 2, WP), dtype=mdt)
    for ht in range(NT):
        xt[:, ht] = xpad[:, :, ht * R : ht * R + R + 2, :]

    # [cout, cin, kh, kw] -> [cout_chunk, cin, kh*kw, cout_slice], contiguous
    wt = np.ascontiguousarray(
        W.reshape(NCH, 128, CIN, KH * KW).transpose(0, 2, 3, 1)
    ).astype(mdt)
    bh = np.ascontiguousarray(b.reshape(NCH, 128).T)

    return [
        {
            "x": xt[core * BPC : (core + 1) * BPC],
            "wt": wt,
            "bias": bh,
        }
        for core in range(NCORES)
    ]


def kernel(x, W, b):
    from concourse.bass_utils import run_bass_kernel_spmd

    if MM_DTYPE not in _cache:
        _cache[MM_DTYPE] = _build(MM_DTYPE)
    nc = _cache[MM_DTYPE]

    in_maps = _make_in_maps(x, W, b)
    try:
        res = run_bass_kernel_spmd(nc, in_maps, list(range(NCORES))).results
    except Exception:
        # A prior session can leave the accelerator in a transient
        # unrecoverable state; one retry after re-init clears it.
        import time

        time.sleep(15)
        res = run_bass_kernel_spmd(nc, in_maps, list(range(NCORES))).results
    # [BPC, 128, NCH, H, W] -> [BPC, NCH*128, H, W]
    outs = [
        res[i]["out"].transpose(0, 2, 1, 3, 4).reshape(BPC, COUT, H, W_)
    for i in range(NCORES)
    ]
    return np.concatenate(outs, axis=0)


# revision 33
# speedup vs baseline: 1.0078x; 1.0078x over previous
"""Trainium2 Bass kernel for nn_Conv2d: x[32,128,56,56] * W[256,128,3,3] + b -> [32,256,56,56].

Stride 1, padding 1, dilation 1. Data-parallel over batch across 8 NeuronCores
(4 images per core, no collectives). Per core the conv is one accumulation
group of 9 matmuls per output tile (one per kernel tap):
PSUM[cout_chunk=128, R*56] += matmul(lhsT=Wt[tap][cin, cout_chunk],
rhs=shifted window of the zero-padded input row-block [cin=128, R+2, 58]).
Bias is fused into the PSUM->SBUF drain on the scalar engine.

Matmuls run in bf16 (1 PE cycle/row vs 4 for exact fp32; enables fast weight
load, so the per-matmul weight switch hides under the previous matmul's
streaming). PSUM accumulation and the output stay fp32; measured absmax rel
err is ~2e-3 vs the fp32 reference (tolerance 2e-2).

Hard-won scheduling facts baked in here (each measured on hardware):
- Each group's 9 matmuls must stay consecutive, on one PSUM bank, with the
  natural [128, R, 56] output AP. Flattening the AP to [128, 448] or
  alternating PSUM banks/rhs tiles between matmuls costs ~40ns on every
  matmul (~20% of its 191ns pitch).
- x row-tile DMAs are issued just-in-time inside the main loop (prefetch
  depth 5), interleaved with output DMAs. The Sync queue triggers DMAs in
  order through an 8-slot completion window, so bulk-issuing all 28 input
  tiles up front parks every output DMA behind ~8MB of input traffic ->
  output SBUF buffers never recycle -> PSUM fills -> the PE stalls mid-run
  and the HAM clock gate re-throttles it (measured 9us stall + 10us at half
  clock).
- A 10-matmul warm-up group on zeroed SBUF (result never read) runs during
  the initial DMA wait so the HAM activity monitor flips the PE to full
  clock (2.4GHz, not the cold 1.2GHz) before/just after the first real
  matmul.
- The final tile ships one DMA per cout chunk, and its last chunk runs as
  two half-height groups (same streamed columns, ~free) so the closing
  drain + DMA overlap the final matmuls: tail measured 1.0us vs 2.5us.
  Note the extra 'ph' pool tag costs ~1.1us of epilogue — the split is
  still net positive (~-0.4us), but that trade bounds how far tail
  splitting can be pushed.
- The framework epilogue serially resets every allocated semaphore
  (~115ns each, ~8.7us total) — every extra pool tag, DMA, or engine queue
  used inflates it, which is why this kernel keeps the structure minimal
  (measured: fancier variants gained ~1.4us in flight but paid it all back
  in epilogue).

Self-contained: hardcodes shapes; host-side pre-pads/retiles x and
pre-transposes W so every device DMA is contiguous.
"""

import numpy as np

B, CIN, H, W_ = 32, 128, 56, 56
COUT, KH, KW = 256, 3, 3
NCORES = 8
BPC = B // NCORES          # images per core
R = 8                      # output rows per tile -> matmul free dim R*56 = 448
NT = H // R                # row tiles per image
NTILE = BPC * NT
HP, WP = H + 2, W_ + 2     # padded 58x58
NCH = COUT // 128          # cout chunks (2)

MM_DTYPE = "bfloat16"
XBUFS = 6                  # x-tile ring depth
PREFETCH = 5               # x tiles loaded ahead of consumption

_cache = {}


def _np_mm_dtype():
    if MM_DTYPE == "bfloat16":
        try:
            import ml_dtypes

            return ml_dtypes.bfloat16
        except ImportError:
            import jax.numpy as jnp

            return np.dtype(jnp.bfloat16)
    return np.float32


def _build(mm_dtype_name):
    import concourse.mybir as mybir
    import concourse.tile as tile
    from concourse import bacc

    dt = mybir.dt
    mmdt = getattr(dt, mm_dtype_name)

    nc = bacc.Bacc("TRN2", target_bir_lowering=False, debug=False)

    # x arrives host-pre-padded per row-tile: [image, row_tile, cin, R+2, 58]
    # (zero border baked in, halo rows duplicated) so every x DMA is one
    # fully contiguous copy and the kernel needs no memsets.
    x_d = nc.dram_tensor(
        "x", [BPC, NT, CIN, R + 2, WP], mmdt, kind="ExternalInput"
    )
    # [chunk, cin, tap, cout_slice]: one contiguous DMA per cout chunk
    wt_d = nc.dram_tensor("wt", [NCH, CIN, KH * KW, 128], mmdt, kind="ExternalInput")
    b_d = nc.dram_tensor("bias", [128, NCH], dt.float32, kind="ExternalInput")
    # Output laid out [image, cout%128 (partition), cout//128, h, w] so both
    # cout chunks of one row-tile go out in a single DMA; host untangles.
    o_d = nc.dram_tensor(
        "out", [BPC, 128, NCH, H, W_], dt.float32, kind="ExternalOutput"
    )

    with tile.TileContext(nc) as tc:
        with (
            tc.tile_pool(name="const", bufs=1) as const_pool,
            tc.tile_pool(name="xin", bufs=XBUFS) as xin_pool,
            tc.tile_pool(name="outp", bufs=4) as out_pool,
            tc.tile_pool(name="psum", bufs=6, space="PSUM") as psum_pool,
        ):
            xt = []

            def load_x(idx):
                n, ht = divmod(idx, NT)
                t = xin_pool.tile([CIN, R + 2, WP], mmdt, tag="xt")
                nc.sync.dma_start(t[:], x_d[n, ht])
                xt.append(t)

            # PE clock warm-up (see module docstring).
            zw_t = const_pool.tile([CIN, 128], mmdt)
            nc.gpsimd.memset(zw_t[:], 0.0)
            zx_t = const_pool.tile([CIN, R, W_], mmdt)
            nc.gpsimd.memset(zx_t[:], 0.0)
            pw = psum_pool.tile([128, R, W_], dt.float32, tag="ps")
            for i in range(11):
                nc.tensor.matmul(
                    pw[:],
                    zw_t[:],
                    zx_t[:],
                    start=(i == 0),
                    stop=(i == 10),
                )

            # Critical path first: the first x tile (the startup gater), then
            # tap-0 of chunk-0 weights (all the first matmul needs), then the
            # rest of the constants and the prefetch window. (A head-split
            # mirror of the tail split was measured NET-NEGATIVE: it starts
            # real work 0.5us earlier but N=224 half-groups can't hide the
            # 97ns weight load under 93ns of streaming, growing the matmul
            # region ~1us.)
            load_x(0)
            w_t = const_pool.tile([CIN, NCH, KH * KW, 128], mmdt)
            nc.sync.dma_start(w_t[:, 0, 0], wt_d[0, :, 0])
            nc.sync.dma_start(w_t[:, 0, 1:], wt_d[0, :, 1:])
            nc.sync.dma_start(w_t[:, 1], wt_d[1])
            b_t = const_pool.tile([128, NCH], dt.float32)
            nc.sync.dma_start(b_t[:], b_d[:])
            for i in range(1, PREFETCH):
                load_x(i)

            for idx in range(NTILE):
                n, ht = divmod(idx, NT)
                if idx + PREFETCH < NTILE:
                    load_x(idx + PREFETCH)
                t = xt[idx]
                ot = out_pool.tile([128, NCH, R, W_], dt.float32, tag="ot")
                for c in range(NCH):
                    if idx == NTILE - 1 and c == 1:
                        # The very last group runs as two half-height pieces
                        # (same total streamed columns, ~free) so the final
                        # drain + output DMA overlap the last matmuls instead
                        # of serializing after them.
                        hr = R // 2
                        for half in range(2):
                            ph = psum_pool.tile(
                                [128, hr, W_], dt.float32, tag="ph", bufs=2
                            )
                            hb = half * hr
                            for kh in range(KH):
                                for kw in range(KW):
                                    pos = kh * KW + kw
                                    nc.tensor.matmul(
                                        ph[:],
                                        w_t[:, 1, pos],
                                        t[:, hb + kh : hb + kh + hr, kw : kw + W_],
                                        start=(pos == 0),
                                        stop=(pos == KH * KW - 1),
                                    )
                            nc.scalar.activation(
                                ot[:, 1, hb : hb + hr],
                                ph[:],
                                mybir.ActivationFunctionType.Identity,
                                bias=b_t[:, 1:2],
                            )
                            nc.sync.dma_start(
                                o_d[n, :, 1, ht * R + hb : ht * R + hb + hr, :],
                                ot[:, 1, hb : hb + hr],
                            )
                        continue
                    p = psum_pool.tile([128, R, W_], dt.float32, tag="ps")
                    for kh in range(KH):
                        for kw in range(KW):
                            pos = kh * KW + kw
                            nc.tensor.matmul(
                                p[:],
                                w_t[:, c, pos],
                                t[:, kh : kh + R, kw : kw + W_],
                                start=(pos == 0),
                                stop=(pos == KH * KW - 1),
                            )
                    nc.scalar.activation(
                        ot[:, c],
                        p[:],
                        mybir.ActivationFunctionType.Identity,
                        bias=b_t[:, c : c + 1],
                    )
                    if idx == NTILE - 1:
                        # Tail latency: ship each chunk of the final tile as
                        # soon as its drain finishes instead of waiting for
                        # both.
                        nc.sync.dma_start(
                            o_d[n, :, c, ht * R : ht * R + R, :],
                            ot[:, c],
                        )
                if idx < NTILE - 1:
                    nc.sync.dma_start(
                        o_d[n, :, :, ht * R : ht * R + R, :],
                        ot[:],
                    )

    nc.compile()
    return nc


def _make_in_maps(x, W, b):
    mdt = _np_mm_dtype()
    x = np.asarray(x, dtype=np.float32)
    W = np.asarray(W, dtype=np.float32)
    b = np.asarray(b, dtype=np.float32)

    # Pre-pad and re-tile x: [B, CIN, 56, 56] -> [B, NT, CIN, R+2, 58] where
    # row-tile ht holds padded rows h0..h0+R+1 (zero border baked in).
    xpad = np.zeros((B, CIN, HP, WP), dtype=mdt)
    xpad[:, :, 1 : H + 1, 1 : W_ + 1] = x.astype(mdt)
    xt = np.empty((B, NT, CIN, R + 2, WP), dtype=mdt)
    for ht in range(NT):
        xt[:, ht] = xpad[:, :, ht * R : ht * R + R + 2, :]

    # [cout, cin, kh, kw] -> [cout_chunk, cin, kh*kw, cout_slice], contiguous
    wt = np.ascontiguousarray(
        W.reshape(NCH, 128, CIN, KH * KW).transpose(0, 2, 3, 1)
    ).astype(mdt)
    bh = np.ascontiguousarray(b.reshape(NCH, 128).T)

    return [
        {
            "x": xt[core * BPC : (core + 1) * BPC],
            "wt": wt,
            "bias": bh,
        }
        for core in range(NCORES)
    ]


def kernel(x, W, b):
    from concourse.bass_utils import run_bass_kernel_spmd

    if MM_DTYPE not in _cache:
        _cache[MM_DTYPE] = _build(MM_DTYPE)
    nc = _cache[MM_DTYPE]

    in_maps = _make_in_maps(x, W, b)
    try:
        res = run_bass_kernel_spmd(nc, in_maps, list(range(NCORES))).results
    except Exception:
        # A prior session can leave the accelerator in a transient
        # unrecoverable state; one retry after re-init clears it.
        import time

        time.sleep(15)
        res = run_bass_kernel_spmd(nc, in_maps, list(range(NCORES))).results
    # [BPC, 128, NCH, H, W] -> [BPC, NCH*128, H, W]
    outs = [
        res[i]["out"].transpose(0, 2, 1, 3, 4).reshape(BPC, COUT, H, W_)
    for i in range(NCORES)
    ]
    return np.concatenate(outs, axis=0)
